# revision 13
# baseline (speedup 1.0000x reference)
"""DecoderTreeLSTMCell Trainium2 Bass kernel.

Strategy: data-parallel over nodes on 8 cores (4096 nodes/core). On the host,
each core's nodes are grouped by `pos` (10 groups) and within each group
ordered [mask=0 | mask=1], each side sub-ordered [depth!=1,2 | d==1 | d==2],
with padded compile-time capacities. fp32r (full fp32 bits, 4x PE streaming
rate) is used for the matmul operands.

All per-node inputs are packed into ONE feature-major tensor AIN [128, Lin]
with per-chunk blocks [child_h(C) | child_c(C) | extras(E)], and outputs into
ONE tensor OUT [128, Lout] with blocks [h_new(M0) | c_new(M0) | c_red(C-M0)].
Chunks are loaded/stored in multi-chunk slabs (one DMA each) because each
dma_start costs ~0.6us of serialized HWDGE time.

Per chunk the device computes: h_cat = child_h (+ extras on the depth
sub-ranges, no masks needed), u = W_f[pos].T @ h_cat over all C columns,
f = sigmoid(u + b_f[pos]), c_red = f * child_c. For the mask=0 columns only
it also computes the i/o/uu matmuls, gates, and c_new/h_new. c is stored as
[c_new | c_red] directly (no blend ops). h rows with mask=1 equal h_prev
exactly and are filled host-side during unshard (data routing only - all
arithmetic happens on device).

The reference computes all 10 pos-matmuls for every node and selects; this
kernel computes only the matmul each node needs, placing it near the DMA
roofline.
"""
import numpy as np

import concourse.bacc as bacc
import concourse.mybir as mybir
from concourse.tile import TileContext
from concourse.bass_utils import run_bass_kernel_spmd

N = 32768
H = 128
N_POS = 10
NC = 8
SH = N // NC  # nodes per core

F32 = mybir.dt.float32
F32R = mybir.dt.float32r
Sig = mybir.ActivationFunctionType.Sigmoid
Tanh = mybir.ActivationFunctionType.Tanh

SLAB_CHUNKS = 2  # chunks per DMA slab

# module-level stash for test harness introspection
LAST = {}


def _roundup(x, m):
    return ((x + m - 1) // m) * m


def _plan(pos, depth, mask):
    """Compute per-core slot layout and DMA packing.

    Returns (chunks, slabs, L, Lin, Lout, slot_idx, ain_slot, ain_kind,
    out_slot, out_kind).

    chunks: (p, off, C, M0, e_ranges, ain_off, out_off) - e_ranges are
    (lo, hi) chunk-relative h-column ranges needing the extras add; the
    packed extras for them sit at ain cols [ain_off+2C ...] sequentially.
    slabs: (ain_off, ain_len, out_off, out_len, [chunk indices]).
    slot_idx: [NC, L] original node index per slot (-1 = pad).
    ain_slot/ain_kind: [Lin] mapping of AIN columns to (slot, kind)
    with kind 0=child_h 1=child_c 2=extras. out_slot/out_kind: [Lout]
    mapping of OUT columns, kind 0=h_new 1=c_new 2=c_red.
    """
    dcl = np.where(depth == 1, 1, np.where(depth == 2, 2, 0))
    idx = {}
    counts = np.zeros((NC, N_POS, 2, 3), np.int64)
    for c in range(NC):
        lo, hi = c * SH, (c + 1) * SH
        pc, dc, mk = pos[lo:hi], dcl[lo:hi], mask[lo:hi]
        for p in range(N_POS):
            for m in range(2):
                for k in range(3):
                    ii = np.nonzero((pc == p) & (mk == m) & (dc == k))[0] + lo
                    idx[(c, p, m, k)] = ii
                    counts[c, p, m, k] = len(ii)

    caps = np.zeros((N_POS, 2, 3), np.int64)
    for p in range(N_POS):
        for m in range(2):
            for k in range(3):
                caps[p, m, k] = _roundup(int(counts[:, p, m, k].max()), 8)

    def emit(p, off, span_lo, span_hi, m0_hi, espans, out):
        # split [span_lo, span_hi) into <=512 pieces; m0_hi marks the end of
        # the full-pipeline (mask=0) region in pos-block coordinates
        start = span_lo
        while start < span_hi:
            end = min(start + 512, span_hi)
            C = end - start
            M0 = min(max(m0_hi - start, 0), C)
            e = []
            for (lo, hi) in espans:
                l2, h2 = max(lo, start), min(hi, end)
                if l2 < h2:
                    e.append((l2 - start, h2 - start))
            out.append((p, off + start, C, M0, e))
            start = end

    raw_chunks = []  # (p, off, C, M0, e_ranges)
    sub_off = np.zeros((N_POS, 2, 3), np.int64)
    off = 0
    for p in range(N_POS):
        m0n = int(caps[p, 0].sum())
        m1n = int(caps[p, 1].sum())
        M0 = max(m0n, 256) if m0n > 0 else 0  # >=256 keeps fp32r at 1cyc/col
        w0 = int(caps[p, 0, 1] + caps[p, 0, 2])
        w1 = int(caps[p, 1, 1] + caps[p, 1, 2])
        for k in range(3):
            sub_off[p, 0, k] = off + int(caps[p, 0, :k].sum())
            sub_off[p, 1, k] = off + M0 + int(caps[p, 1, :k].sum())
        espans = []
        if w0:
            espans.append((m0n - w0, m0n))
        if w1:
            espans.append((M0 + m1n - w1, M0 + m1n))
        if M0 + m1n <= 512:
            emit(p, off, 0, M0 + m1n, M0, espans, raw_chunks)
        else:
            # split at the mask0/mask1 boundary (balanced pieces); emit()
            # further subdivides if either side still exceeds 512
            emit(p, off, 0, M0, M0, espans, raw_chunks)
            emit(p, off, M0, M0 + m1n, M0, espans, raw_chunks)
        off += M0 + m1n
    L = off

    slot_idx = np.full((NC, L), -1, np.int64)
    for c in range(NC):
        for p in range(N_POS):
            for m in range(2):
                for k in range(3):
                    ii = idx[(c, p, m, k)]
                    o = int(sub_off[p, m, k])
                    slot_idx[c, o:o + len(ii)] = ii

    # packing: AIN blocks [h(C) | c(C) | e(E)], OUT blocks [h_new | c_new | cr]
    chunks = []
    ain_slot, ain_kind, out_slot, out_kind = [], [], [], []
    a = 0
    o = 0
    for (p, off, C, M0, e_ranges) in raw_chunks:
        chunks.append((p, off, C, M0, e_ranges, a, o))
        ain_slot.extend(range(off, off + C)); ain_kind.extend([0] * C)
        ain_slot.extend(range(off, off + C)); ain_kind.extend([1] * C)
        for (lo, hi) in e_ranges:
            ain_slot.extend(range(off + lo, off + hi))
            ain_kind.extend([2] * (hi - lo))
        a += 2 * C + sum(hi - lo for lo, hi in e_ranges)
        if M0 > 0:
            out_slot.extend(range(off, off + M0)); out_kind.extend([0] * M0)
            out_slot.extend(range(off, off + M0)); out_kind.extend([1] * M0)
        if C > M0:
            out_slot.extend(range(off + M0, off + C))
            out_kind.extend([2] * (C - M0))
        o += M0 + C
    Lin, Lout = a, o

    slabs = []
    for s in range(0, len(chunks), SLAB_CHUNKS):
        grp = list(range(s, min(s + SLAB_CHUNKS, len(chunks))))
        a0 = chunks[grp[0]][5]
        o0 = chunks[grp[0]][6]
        last = chunks[grp[-1]]
        a1 = last[5] + 2 * last[2] + sum(hi - lo for lo, hi in last[4])
        o1 = last[6] + last[3] + last[2]
        slabs.append((a0, a1 - a0, o0, o1 - o0, grp))

    return (chunks, slabs, L, Lin, Lout, slot_idx,
            np.array(ain_slot), np.array(ain_kind),
            np.array(out_slot), np.array(out_kind))


def _build(plan, reps=1):
    chunks, slabs, L, Lin, Lout = plan[:5]
    nc = bacc.Bacc("TRN2", target_bir_lowering=False)
    AIN = nc.dram_tensor("AIN", [H, Lin], F32R, kind="ExternalInput")
    W = nc.dram_tensor("W", [H, N_POS * 4 * H], F32R, kind="ExternalInput")
    BIAS = nc.dram_tensor("BIAS", [H, 13], F32, kind="ExternalInput")
    OUT = nc.dram_tensor("OUT", [H, Lout], F32, kind="ExternalOutput")

    with TileContext(nc) as tc:
        with (
            tc.tile_pool(name="const", bufs=1) as cpool,
            tc.tile_pool(name="io", bufs=5) as io,
            tc.tile_pool(name="wk", bufs=4) as wk,
            tc.tile_pool(name="ps_u", bufs=2, space="PSUM") as ps_u,
            tc.tile_pool(name="ps_i", bufs=2, space="PSUM") as ps_i,
            tc.tile_pool(name="ps_o", bufs=2, space="PSUM") as ps_o,
            tc.tile_pool(name="ps_t", bufs=2, space="PSUM") as ps_t,
        ):
            bias_sb = cpool.tile([H, 13], F32, tag="bias")
            nc.sync.dma_start(out=bias_sb[:, :], in_=BIAS[:, :])
            w_tiles = {}

            def w_load(p):
                if p not in w_tiles:
                    t = cpool.tile([H, 4 * H], F32R, tag=f"w{p}")
                    nc.sync.dma_start(
                        out=t[:, :], in_=W[:, p * 4 * H:(p + 1) * 4 * H])
                    w_tiles[p] = t
                return w_tiles[p]

            def body(_iv=None):
                for (a0, alen, o0, olen, grp) in slabs:
                    ain = io.tile([H, alen], F32R, tag="ain")
                    nc.sync.dma_start(out=ain[:, :], in_=AIN[:, a0:a0 + alen])
                    out = io.tile([H, olen], F32, tag="out")

                    # extras adds first (keeps slab-tile write/read ordering
                    # simple for the scheduler)
                    for ci in grp:
                        (p, off, C, M0, e_ranges, ca, co) = chunks[ci]
                        ra = ca - a0
                        eoff = ra + 2 * C
                        for (lo, hi) in e_ranges:
                            w_ = hi - lo
                            nc.vector.tensor_add(
                                ain[:, ra + lo:ra + hi],
                                ain[:, ra + lo:ra + hi],
                                ain[:, eoff:eoff + w_])
                            eoff += w_

                    for ci in grp:
                        (p, off, C, M0, e_ranges, ca, co) = chunks[ci]
                        ra = ca - a0
                        ro = co - o0
                        h_v = ain[:, ra:ra + C]
                        c_v = ain[:, ra + C:ra + 2 * C].bitcast(F32)
                        w_sb = w_load(p)
                        wof = 0

                        p_u = ps_u.tile([H, C], F32, tag="u")
                        nc.tensor.matmul(p_u[:, :], w_sb[:, wof:wof + H],
                                         h_v, start=True, stop=True)
                        f_sb = wk.tile([H, C], F32, tag="f")
                        nc.scalar.activation(f_sb[:, :], p_u[:, :], Sig,
                                             bias=bias_sb[:, p:p + 1])

                        if M0 > 0:
                            cr_sb = wk.tile([H, M0], F32, tag="cr")
                            nc.vector.tensor_mul(cr_sb[:, :], f_sb[:, 0:M0],
                                                 c_v[:, 0:M0])
                            if C > M0:
                                nc.vector.tensor_mul(
                                    out[:, ro + 2 * M0:ro + M0 + C],
                                    f_sb[:, M0:C], c_v[:, M0:C])

                            p_i = ps_i.tile([H, M0], F32, tag="i")
                            nc.tensor.matmul(p_i[:, :],
                                             w_sb[:, wof + H:wof + 2 * H],
                                             h_v[:, 0:M0], start=True,
                                             stop=True)
                            p_o = ps_o.tile([H, M0], F32, tag="o")
                            nc.tensor.matmul(p_o[:, :],
                                             w_sb[:, wof + 2 * H:wof + 3 * H],
                                             h_v[:, 0:M0], start=True,
                                             stop=True)
                            p_t = ps_t.tile([H, M0], F32, tag="t")
                            nc.tensor.matmul(p_t[:, :],
                                             w_sb[:, wof + 3 * H:wof + 4 * H],
                                             h_v[:, 0:M0], start=True,
                                             stop=True)

                            si_sb = wk.tile([H, M0], F32, tag="si")
                            nc.scalar.activation(si_sb[:, :], p_i[:, :], Sig,
                                                 bias=bias_sb[:, 10:11])
                            tu_sb = wk.tile([H, M0], F32, tag="tu")
                            nc.scalar.activation(tu_sb[:, :], p_t[:, :], Tanh,
                                                 bias=bias_sb[:, 12:13])
                            nc.vector.tensor_mul(si_sb[:, :], si_sb[:, :],
                                                 tu_sb[:, :])
                            c_new = out[:, ro + M0:ro + 2 * M0]
                            nc.vector.tensor_add(c_new, si_sb[:, :],
                                                 cr_sb[:, :])

                            so_sb = wk.tile([H, M0], F32, tag="so")
                            nc.scalar.activation(so_sb[:, :], p_o[:, :], Sig,
                                                 bias=bias_sb[:, 11:12])
                            th_sb = wk.tile([H, M0], F32, tag="th")
                            nc.scalar.activation(th_sb[:, :], c_new, Tanh)
                            nc.vector.tensor_mul(out[:, ro:ro + M0],
                                                 so_sb[:, :], th_sb[:, :])
                        else:
                            # u-only chunk: c_red straight into OUT block
                            nc.vector.tensor_mul(out[:, ro:ro + C],
                                                 f_sb[:, :], c_v)

                    nc.gpsimd.dma_start(out=OUT[:, o0:o0 + olen], in_=out[:, :])

            if reps == 1:
                body()
            else:
                for p_ in range(N_POS):
                    w_load(p_)
                with tc.For_i(0, reps, 1) as _i:
                    body(_i)
    nc.finalize()
    return nc


_BUILD_CACHE = {}


def _prepare(inputs, reps=1):
    global N, H, N_POS, SH
    N, _, H = np.asarray(inputs["child_h"]).shape
    N_POS = np.asarray(inputs["W_f"]).shape[0] // H
    SH = N // NC
    child_h = np.asarray(inputs["child_h"], np.float32).reshape(N, H)
    child_c = np.asarray(inputs["child_c"], np.float32).reshape(N, H)
    e1 = np.asarray(inputs["extra_input_depth_1"], np.float32)
    e2 = np.asarray(inputs["extra_input_depth_2"], np.float32)
    h_prev = np.asarray(inputs["h_prev"], np.float32)
    pos = np.asarray(inputs["pos"]).astype(np.int64)
    depth = np.asarray(inputs["depth"]).astype(np.int64)
    mask = np.asarray(inputs["mask"]).astype(np.int64)
    W_f = np.asarray(inputs["W_f"], np.float32)
    b_f = np.asarray(inputs["b_f"], np.float32)
    W_iou = np.asarray(inputs["W_iou"], np.float32)
    b_iou = np.asarray(inputs["b_iou"], np.float32)

    mask01 = (mask != 0).astype(np.int64)
    plan = _plan(pos, depth, mask01)
    (chunks, slabs, L, Lin, Lout, slot_idx,
     ain_slot, ain_kind, out_slot, out_kind) = plan

    key = (tuple((p, o, C, M0, tuple(e), ca, co)
                 for p, o, C, M0, e, ca, co in chunks), Lin, Lout, reps)
    if key not in _BUILD_CACHE:
        _BUILD_CACHE[key] = _build(plan, reps=reps)
    nc = _BUILD_CACHE[key]

    # weights packed [H, 10*4*H]: per pos p: [W_f_p | Wi0^T | Wi1^T | Wi2^T]
    Wp = np.empty((H, N_POS * 4 * H), np.float32)
    W_f_r = W_f.reshape(N_POS, H, H)
    for p in range(N_POS):
        base = p * 4 * H
        Wp[:, base:base + H] = W_f_r[p]
        for j in range(3):
            Wp[:, base + (j + 1) * H:base + (j + 2) * H] = \
                W_iou[j * H:(j + 1) * H, p * H:(p + 1) * H].T
    bias = np.empty((H, 13), np.float32)
    bias[:, :N_POS] = b_f.reshape(N_POS, H).T
    bias[:, 10] = b_iou[0, 0:H]
    bias[:, 11] = b_iou[0, H:2 * H]
    bias[:, 12] = b_iou[0, 2 * H:3 * H]

    # e source per node: e1 where depth==1, e2 where depth==2 (others unused)
    e_src = np.where((depth == 1)[:, None], e1, e2).astype(np.float32)
    srcs = (child_h, child_c, e_src)

    in_maps = []
    for c in range(NC):
        node = slot_idx[c][ain_slot]          # [Lin] node per ain col, -1 pad
        AIN = np.zeros((H, Lin), np.float32)
        for kind in range(3):
            m = (ain_kind == kind) & (node >= 0)
            AIN[:, m] = srcs[kind][node[m]].T
        in_maps.append({"AIN": AIN, "W": Wp, "BIAS": bias})

    mask_on = mask != 0

    def assemble(results):
        h = np.empty((N, H), np.float32)
        cc = np.empty((N, H), np.float32)
        for c in range(NC):
            node = slot_idx[c][out_slot]      # [Lout] node per out col
            O = results[c]["OUT"]
            mh = (out_kind == 0) & (node >= 0)
            h[node[mh]] = O[:, mh].T
            mc = (out_kind != 0) & (node >= 0)
            cc[node[mc]] = O[:, mc].T
        h[mask_on] = h_prev[mask_on]
        return h, cc

    return nc, in_maps, assemble


def kernel(**inputs):
    nc, in_maps, assemble = _prepare(inputs)
    try:
        res = run_bass_kernel_spmd(nc, in_maps, list(range(NC)))
    except Exception:
        # first execution of a freshly compiled NEFF occasionally kills the
        # worker (transient); one retry has always succeeded
        res = run_bass_kernel_spmd(nc, in_maps, list(range(NC)))
    LAST["results"] = res
    LAST["nc"] = nc
    return assemble(res.results)


# revision 14
# speedup vs baseline: 1.0440x; 1.0440x over previous
"""DecoderTreeLSTMCell Trainium2 Bass kernel.

Strategy: data-parallel over nodes on 8 cores (4096 nodes/core). On the host,
each core's nodes are grouped by `pos` (10 groups) and within each group
ordered [mask=0 | mask=1], each side sub-ordered [depth!=1,2 | d==1 | d==2],
with padded compile-time capacities. fp32r (full fp32 bits, 4x PE streaming
rate) is used for the matmul operands.

All per-node inputs are packed into ONE feature-major tensor AIN [128, Lin]
with per-chunk blocks [child_h(C) | child_c(C) | extras(E)], and outputs into
ONE tensor OUT [128, Lout] with blocks [h_new(M0) | c_new(M0) | c_red(C-M0)].
Chunks are loaded/stored in multi-chunk slabs (one DMA each) because each
dma_start costs ~0.6us of serialized HWDGE time.

Per chunk the device computes: h_cat = child_h (+ extras on the depth
sub-ranges, no masks needed), u = W_f[pos].T @ h_cat over all C columns,
f = sigmoid(u + b_f[pos]), c_red = f * child_c. For the mask=0 columns only
it also computes the i/o/uu matmuls, gates, and c_new/h_new. c is stored as
[c_new | c_red] directly (no blend ops). h rows with mask=1 equal h_prev
exactly and are filled host-side during unshard (data routing only - all
arithmetic happens on device).

The reference computes all 10 pos-matmuls for every node and selects; this
kernel computes only the matmul each node needs, placing it near the DMA
roofline.
"""
import numpy as np

import concourse.bacc as bacc
import concourse.mybir as mybir
from concourse.tile import TileContext
from concourse.bass_utils import run_bass_kernel_spmd

N = 32768
H = 128
N_POS = 10
NC = 8
SH = N // NC  # nodes per core

F32 = mybir.dt.float32
F32R = mybir.dt.float32r
Sig = mybir.ActivationFunctionType.Sigmoid
Tanh = mybir.ActivationFunctionType.Tanh

SLAB_CHUNKS = 2  # chunks per DMA slab

# module-level stash for test harness introspection
LAST = {}


def _roundup(x, m):
    return ((x + m - 1) // m) * m


def _plan(pos, depth, mask):
    """Compute per-core slot layout and DMA packing.

    Returns (chunks, slabs, L, Lin, Lout, slot_idx, ain_slot, ain_kind,
    out_slot, out_kind).

    chunks: (p, off, C, M0, e_ranges, ain_off, out_off) - e_ranges are
    (lo, hi) chunk-relative h-column ranges needing the extras add; the
    packed extras for them sit at ain cols [ain_off+2C ...] sequentially.
    slabs: (ain_off, ain_len, out_off, out_len, [chunk indices]).
    slot_idx: [NC, L] original node index per slot (-1 = pad).
    ain_slot/ain_kind: [Lin] mapping of AIN columns to (slot, kind)
    with kind 0=child_h 1=child_c 2=extras. out_slot/out_kind: [Lout]
    mapping of OUT columns, kind 0=h_new 1=c_new 2=c_red.
    """
    dcl = np.where(depth == 1, 1, np.where(depth == 2, 2, 0))
    idx = {}
    counts = np.zeros((NC, N_POS, 2, 3), np.int64)
    for c in range(NC):
        lo, hi = c * SH, (c + 1) * SH
        pc, dc, mk = pos[lo:hi], dcl[lo:hi], mask[lo:hi]
        for p in range(N_POS):
            for m in range(2):
                for k in range(3):
                    ii = np.nonzero((pc == p) & (mk == m) & (dc == k))[0] + lo
                    idx[(c, p, m, k)] = ii
                    counts[c, p, m, k] = len(ii)

    caps = np.zeros((N_POS, 2, 3), np.int64)
    for p in range(N_POS):
        for m in range(2):
            for k in range(3):
                caps[p, m, k] = _roundup(int(counts[:, p, m, k].max()), 8)

    def emit(p, off, span_lo, span_hi, m0_hi, espans, out):
        # split [span_lo, span_hi) into <=512 pieces; m0_hi marks the end of
        # the full-pipeline (mask=0) region in pos-block coordinates
        start = span_lo
        while start < span_hi:
            end = min(start + 512, span_hi)
            C = end - start
            M0 = min(max(m0_hi - start, 0), C)
            e = []
            for (lo, hi) in espans:
                l2, h2 = max(lo, start), min(hi, end)
                if l2 < h2:
                    e.append((l2 - start, h2 - start))
            out.append((p, off + start, C, M0, e))
            start = end

    raw_chunks = []  # (p, off, C, M0, e_ranges)
    sub_off = np.zeros((N_POS, 2, 3), np.int64)
    off = 0
    for p in range(N_POS):
        m0n = int(caps[p, 0].sum())
        m1n = int(caps[p, 1].sum())
        M0 = max(m0n, 256) if m0n > 0 else 0  # >=256 keeps fp32r at 1cyc/col
        w0 = int(caps[p, 0, 1] + caps[p, 0, 2])
        w1 = int(caps[p, 1, 1] + caps[p, 1, 2])
        for k in range(3):
            sub_off[p, 0, k] = off + int(caps[p, 0, :k].sum())
            sub_off[p, 1, k] = off + M0 + int(caps[p, 1, :k].sum())
        espans = []
        if w0:
            espans.append((m0n - w0, m0n))
        if w1:
            espans.append((M0 + m1n - w1, M0 + m1n))
        if M0 + m1n <= 512:
            emit(p, off, 0, M0 + m1n, M0, espans, raw_chunks)
        else:
            # split at the mask0/mask1 boundary (balanced pieces); emit()
            # further subdivides if either side still exceeds 512
            emit(p, off, 0, M0, M0, espans, raw_chunks)
            emit(p, off, M0, M0 + m1n, M0, espans, raw_chunks)
        off += M0 + m1n
    L = off

    slot_idx = np.full((NC, L), -1, np.int64)
    for c in range(NC):
        for p in range(N_POS):
            for m in range(2):
                for k in range(3):
                    ii = idx[(c, p, m, k)]
                    o = int(sub_off[p, m, k])
                    slot_idx[c, o:o + len(ii)] = ii

    # packing: AIN blocks [h(C) | c(C) | e(E)], OUT blocks [h_new | c_new | cr]
    chunks = []
    ain_slot, ain_kind, out_slot, out_kind = [], [], [], []
    a = 0
    o = 0
    for (p, off, C, M0, e_ranges) in raw_chunks:
        chunks.append((p, off, C, M0, e_ranges, a, o))
        ain_slot.extend(range(off, off + C)); ain_kind.extend([0] * C)
        ain_slot.extend(range(off, off + C)); ain_kind.extend([1] * C)
        for (lo, hi) in e_ranges:
            ain_slot.extend(range(off + lo, off + hi))
            ain_kind.extend([2] * (hi - lo))
        a += 2 * C + sum(hi - lo for lo, hi in e_ranges)
        if M0 > 0:
            out_slot.extend(range(off, off + M0)); out_kind.extend([0] * M0)
            out_slot.extend(range(off, off + M0)); out_kind.extend([1] * M0)
        if C > M0:
            out_slot.extend(range(off + M0, off + C))
            out_kind.extend([2] * (C - M0))
        o += M0 + C
    Lin, Lout = a, o

    slabs = []
    for s in range(0, len(chunks), SLAB_CHUNKS):
        grp = list(range(s, min(s + SLAB_CHUNKS, len(chunks))))
        a0 = chunks[grp[0]][5]
        o0 = chunks[grp[0]][6]
        last = chunks[grp[-1]]
        a1 = last[5] + 2 * last[2] + sum(hi - lo for lo, hi in last[4])
        o1 = last[6] + last[3] + last[2]
        slabs.append((a0, a1 - a0, o0, o1 - o0, grp))

    return (chunks, slabs, L, Lin, Lout, slot_idx,
            np.array(ain_slot), np.array(ain_kind),
            np.array(out_slot), np.array(out_kind))


def _build(plan, reps=1):
    chunks, slabs, L, Lin, Lout = plan[:5]
    nc = bacc.Bacc("TRN2", target_bir_lowering=False)
    AIN = nc.dram_tensor("AIN", [H, Lin], F32R, kind="ExternalInput")
    W = nc.dram_tensor("W", [H, N_POS * 4 * H], F32R, kind="ExternalInput")
    BIAS = nc.dram_tensor("BIAS", [H, 13], F32, kind="ExternalInput")
    OUT = nc.dram_tensor("OUT", [H, Lout], F32, kind="ExternalOutput")

    with TileContext(nc) as tc:
        with (
            tc.tile_pool(name="const", bufs=1) as cpool,
            tc.tile_pool(name="io", bufs=5) as io,
            tc.tile_pool(name="wk", bufs=4) as wk,
            tc.tile_pool(name="ps_u", bufs=2, space="PSUM") as ps_u,
            tc.tile_pool(name="ps_i", bufs=2, space="PSUM") as ps_i,
            tc.tile_pool(name="ps_o", bufs=2, space="PSUM") as ps_o,
            tc.tile_pool(name="ps_t", bufs=2, space="PSUM") as ps_t,
        ):
            bias_sb = cpool.tile([H, 13], F32, tag="bias")
            nc.sync.dma_start(out=bias_sb[:, :], in_=BIAS[:, :])
            w_tiles = {}

            def w_load(p):
                if p not in w_tiles:
                    t = cpool.tile([H, 4 * H], F32R, tag=f"w{p}")
                    nc.sync.dma_start(
                        out=t[:, :], in_=W[:, p * 4 * H:(p + 1) * 4 * H])
                    w_tiles[p] = t
                return w_tiles[p]

            def body(_iv=None):
                for (a0, alen, o0, olen, grp) in slabs:
                    ain = io.tile([H, alen], F32R, tag="ain")
                    nc.sync.dma_start(out=ain[:, :], in_=AIN[:, a0:a0 + alen])
                    out = io.tile([H, olen], F32, tag="out")

                    # extras adds first (keeps slab-tile write/read ordering
                    # simple for the scheduler)
                    for ci in grp:
                        (p, off, C, M0, e_ranges, ca, co) = chunks[ci]
                        ra = ca - a0
                        eoff = ra + 2 * C
                        for (lo, hi) in e_ranges:
                            w_ = hi - lo
                            nc.vector.tensor_add(
                                ain[:, ra + lo:ra + hi],
                                ain[:, ra + lo:ra + hi],
                                ain[:, eoff:eoff + w_])
                            eoff += w_

                    for ci in grp:
                        (p, off, C, M0, e_ranges, ca, co) = chunks[ci]
                        ra = ca - a0
                        ro = co - o0
                        h_v = ain[:, ra:ra + C]
                        c_v = ain[:, ra + C:ra + 2 * C].bitcast(F32)
                        w_sb = w_load(p)
                        wof = 0

                        p_u = ps_u.tile([H, C], F32, tag="u")
                        nc.tensor.matmul(p_u[:, :], w_sb[:, wof:wof + H],
                                         h_v, start=True, stop=True)
                        f_sb = wk.tile([H, C], F32, tag="f")
                        nc.scalar.activation(f_sb[:, :], p_u[:, :], Sig,
                                             bias=bias_sb[:, p:p + 1])

                        if M0 > 0:
                            cr_sb = wk.tile([H, M0], F32, tag="cr")
                            nc.vector.tensor_mul(cr_sb[:, :], f_sb[:, 0:M0],
                                                 c_v[:, 0:M0])
                            if C > M0:
                                nc.vector.tensor_mul(
                                    out[:, ro + 2 * M0:ro + M0 + C],
                                    f_sb[:, M0:C], c_v[:, M0:C])

                            p_i = ps_i.tile([H, M0], F32, tag="i")
                            nc.tensor.matmul(p_i[:, :],
                                             w_sb[:, wof + H:wof + 2 * H],
                                             h_v[:, 0:M0], start=True,
                                             stop=True)
                            p_o = ps_o.tile([H, M0], F32, tag="o")
                            nc.tensor.matmul(p_o[:, :],
                                             w_sb[:, wof + 2 * H:wof + 3 * H],
                                             h_v[:, 0:M0], start=True,
                                             stop=True)
                            p_t = ps_t.tile([H, M0], F32, tag="t")
                            nc.tensor.matmul(p_t[:, :],
                                             w_sb[:, wof + 3 * H:wof + 4 * H],
                                             h_v[:, 0:M0], start=True,
                                             stop=True)

                            si_sb = wk.tile([H, M0], F32, tag="si")
                            nc.scalar.activation(si_sb[:, :], p_i[:, :], Sig,
                                                 bias=bias_sb[:, 10:11])
                            tu_sb = wk.tile([H, M0], F32, tag="tu")
                            nc.scalar.activation(tu_sb[:, :], p_t[:, :], Tanh,
                                                 bias=bias_sb[:, 12:13])
                            nc.vector.tensor_mul(si_sb[:, :], si_sb[:, :],
                                                 tu_sb[:, :])
                            c_new = out[:, ro + M0:ro + 2 * M0]
                            nc.vector.tensor_add(c_new, si_sb[:, :],
                                                 cr_sb[:, :])

                            so_sb = wk.tile([H, M0], F32, tag="so")
                            nc.scalar.activation(so_sb[:, :], p_o[:, :], Sig,
                                                 bias=bias_sb[:, 11:12])
                            th_sb = wk.tile([H, M0], F32, tag="th")
                            nc.scalar.activation(th_sb[:, :], c_new, Tanh)
                            nc.vector.tensor_mul(out[:, ro:ro + M0],
                                                 so_sb[:, :], th_sb[:, :])
                        else:
                            # u-only chunk: c_red straight into OUT block
                            nc.vector.tensor_mul(out[:, ro:ro + C],
                                                 f_sb[:, :], c_v)

                    nc.sync.dma_start(out=OUT[:, o0:o0 + olen], in_=out[:, :])

            if reps == 1:
                body()
            else:
                for p_ in range(N_POS):
                    w_load(p_)
                with tc.For_i(0, reps, 1) as _i:
                    body(_i)
    nc.finalize()
    return nc


_BUILD_CACHE = {}


def _prepare(inputs, reps=1):
    global N, H, N_POS, SH
    N, _, H = np.asarray(inputs["child_h"]).shape
    N_POS = np.asarray(inputs["W_f"]).shape[0] // H
    SH = N // NC
    child_h = np.asarray(inputs["child_h"], np.float32).reshape(N, H)
    child_c = np.asarray(inputs["child_c"], np.float32).reshape(N, H)
    e1 = np.asarray(inputs["extra_input_depth_1"], np.float32)
    e2 = np.asarray(inputs["extra_input_depth_2"], np.float32)
    h_prev = np.asarray(inputs["h_prev"], np.float32)
    pos = np.asarray(inputs["pos"]).astype(np.int64)
    depth = np.asarray(inputs["depth"]).astype(np.int64)
    mask = np.asarray(inputs["mask"]).astype(np.int64)
    W_f = np.asarray(inputs["W_f"], np.float32)
    b_f = np.asarray(inputs["b_f"], np.float32)
    W_iou = np.asarray(inputs["W_iou"], np.float32)
    b_iou = np.asarray(inputs["b_iou"], np.float32)

    mask01 = (mask != 0).astype(np.int64)
    plan = _plan(pos, depth, mask01)
    (chunks, slabs, L, Lin, Lout, slot_idx,
     ain_slot, ain_kind, out_slot, out_kind) = plan

    key = (tuple((p, o, C, M0, tuple(e), ca, co)
                 for p, o, C, M0, e, ca, co in chunks), Lin, Lout, reps)
    if key not in _BUILD_CACHE:
        _BUILD_CACHE[key] = _build(plan, reps=reps)
    nc = _BUILD_CACHE[key]

    # weights packed [H, 10*4*H]: per pos p: [W_f_p | Wi0^T | Wi1^T | Wi2^T]
    Wp = np.empty((H, N_POS * 4 * H), np.float32)
    W_f_r = W_f.reshape(N_POS, H, H)
    for p in range(N_POS):
        base = p * 4 * H
        Wp[:, base:base + H] = W_f_r[p]
        for j in range(3):
            Wp[:, base + (j + 1) * H:base + (j + 2) * H] = \
                W_iou[j * H:(j + 1) * H, p * H:(p + 1) * H].T
    bias = np.empty((H, 13), np.float32)
    bias[:, :N_POS] = b_f.reshape(N_POS, H).T
    bias[:, 10] = b_iou[0, 0:H]
    bias[:, 11] = b_iou[0, H:2 * H]
    bias[:, 12] = b_iou[0, 2 * H:3 * H]

    # e source per node: e1 where depth==1, e2 where depth==2 (others unused)
    e_src = np.where((depth == 1)[:, None], e1, e2).astype(np.float32)
    srcs = (child_h, child_c, e_src)

    in_maps = []
    for c in range(NC):
        node = slot_idx[c][ain_slot]          # [Lin] node per ain col, -1 pad
        AIN = np.zeros((H, Lin), np.float32)
        for kind in range(3):
            m = (ain_kind == kind) & (node >= 0)
            AIN[:, m] = srcs[kind][node[m]].T
        in_maps.append({"AIN": AIN, "W": Wp, "BIAS": bias})

    mask_on = mask != 0

    def assemble(results):
        h = np.empty((N, H), np.float32)
        cc = np.empty((N, H), np.float32)
        for c in range(NC):
            node = slot_idx[c][out_slot]      # [Lout] node per out col
            O = results[c]["OUT"]
            mh = (out_kind == 0) & (node >= 0)
            h[node[mh]] = O[:, mh].T
            mc = (out_kind != 0) & (node >= 0)
            cc[node[mc]] = O[:, mc].T
        h[mask_on] = h_prev[mask_on]
        return h, cc

    return nc, in_maps, assemble


def kernel(**inputs):
    nc, in_maps, assemble = _prepare(inputs)
    try:
        res = run_bass_kernel_spmd(nc, in_maps, list(range(NC)))
    except Exception:
        # first execution of a freshly compiled NEFF occasionally kills the
        # worker (transient); one retry has always succeeded
        res = run_bass_kernel_spmd(nc, in_maps, list(range(NC)))
    LAST["results"] = res
    LAST["nc"] = nc
    return assemble(res.results)
